# revision 11
# baseline (speedup 1.0000x reference)
"""Trainium2 Bass kernel for nn_BranchingQNetwork (12-branch dueling Q-MLP).

Strategy: data-parallel over batch (8 cores x 1024 rows). Per core, all 12
branch MLPs run as feature-major GEMM chains in bf16 (weights stationary,
activations streaming, fp32 PSUM accumulate). bf16 weight tiles get the
fast-weight-load path (FWL), so each 128x128 LDWEIGHTS (~50-100ns) hides
behind its 512-column MATMUL (~216ns) — the fp32r baseline was
LDWEIGHTS-limited. Per-branch weights are SBUF-resident as per-k tiles
(prefetched one branch ahead, exact DMA deps), so W2 is read from HBM once
per branch instead of once per batch tile. Layer 1 has K=62 < 128, so its
m-tile pairs run concurrently in disjoint 64-row PE array groups
(tile_position row packing with px duplicated into partitions 64..125),
halving L1 column time. The dueling head (v + a - mean(a)) is linear and
folded into a single [512, 11] matrix on the host; it runs with Wq
stationary producing [11, batch]-major output, transposed on the host.
"""
import sys

sys.path.insert(0, "/opt/trn_rl_repo")

import numpy as np
import ml_dtypes

# problem dims (hardcoded per harness contract)
B = 8192
OBS = 249
NB = 12
NA = 11
NODE = 45
GRP = 17
D0 = 62
D1 = 2048
D2 = 1024
D3 = 512

NCORES = 8
LB = B // NCORES     # local batch per core
BT = 512             # batch tile
NBT = LB // BT
M1 = D1 // 128       # 16 output tiles of layer 1
MP1 = M1 // 2        # 8 row-packed L1 pairs
K2 = D1 // 128       # 16 contraction tiles of layer 2
M2 = D2 // 128       # 8
K3 = D2 // 128       # 8
M3 = D3 // 128       # 4
KH = D3 // 128       # 4
NAP = 12             # head width padded even

BF16 = ml_dtypes.bfloat16

_NC_CACHE = {}
LAST_RESULT = None


def _build_nc():
    if "nc" in _NC_CACHE:
        return _NC_CACHE["nc"]
    from concourse import bacc
    import concourse.mybir as mybir
    import concourse.tile as tile

    f32 = mybir.dt.float32
    bf16 = mybir.dt.bfloat16
    Relu = mybir.ActivationFunctionType.Relu
    ADD = mybir.AluOpType.add
    MAX = mybir.AluOpType.max

    nc = bacc.Bacc("TRN2")

    xT_d = nc.declare_dram_parameter("xT", [OBS, LB], bf16, isOutput=False)
    W1_d = nc.declare_dram_parameter("W1p", [NB, 128, D1], bf16, isOutput=False)
    W2_d = nc.declare_dram_parameter("W2p", [NB, K2, 128, D2], bf16, isOutput=False)
    W3_d = nc.declare_dram_parameter("W3p", [NB, K3, 128, D3], bf16, isOutput=False)
    Wq_d = nc.declare_dram_parameter("Wqp", [NB, KH, 128, NAP], bf16, isOutput=False)
    b_d = nc.declare_dram_parameter("bp", [NB, 128, M1 + M2 + M3], f32, isOutput=False)
    bq_d = nc.declare_dram_parameter("bqp", [NB, NAP, 1], f32, isOutput=False)
    out_d = nc.declare_dram_parameter("out", [NB, NA, LB], f32, isOutput=True)

    with tile.TileContext(nc) as tc:
        with (
            tc.tile_pool(name="wp1", bufs=2) as wp1,
            tc.tile_pool(name="wp2", bufs=2 * K2) as wp2,
            tc.tile_pool(name="wp3", bufs=2 * K3) as wp3,
            tc.tile_pool(name="wpq", bufs=2) as wpq,
            tc.tile_pool(name="bbp", bufs=2) as bbp,
            tc.tile_pool(name="pxp", bufs=3) as pxp,
            tc.tile_pool(name="actp", bufs=1) as actp,
            tc.tile_pool(name="osp", bufs=3) as osp,
            tc.tile_pool(name="psp", bufs=8, space="PSUM") as psp,
        ):
            h1 = actp.tile([128, K2, BT], bf16, tag="h1")
            h2 = actp.tile([128, K3, BT], bf16, tag="h2")
            h3 = actp.tile([128, KH, BT], bf16, tag="h3")

            iters = [(br, bt) for br in range(NB) for bt in range(NBT)]
            loaded = {}
            pxs = {}

            def load_branch(br):
                w1t = wp1.tile([128, D1], bf16, tag="w1", name=f"w1_{br}")
                btile = bbp.tile([128, M1 + M2 + M3], f32, tag="b", name=f"b_{br}")
                bqt = bbp.tile([NAP, 1], f32, tag="bq", name=f"bq_{br}")
                wqt = wpq.tile([128, KH, NAP], bf16, tag="wq", name=f"wq_{br}")
                if br == 0:
                    # chunk W1[0] across both queues so the first L1 pair can
                    # start as soon as px + the first 512 columns have landed
                    nc.scalar.dma_start(btile[:], b_d[br])
                    nc.scalar.dma_start(bqt[:], bq_d[br])
                    nc.scalar.dma_start(wqt[:], Wq_d[br].rearrange("k p a -> p k a"))
                    for c in range(4):
                        eng = (nc.sync, nc.scalar)[c % 2]
                        csl = slice(c * 512, (c + 1) * 512)
                        eng.dma_start(w1t[:, csl], W1_d[br, :, csl])
                else:
                    nc.scalar.dma_start(w1t[:], W1_d[br])
                    nc.scalar.dma_start(btile[:], b_d[br])
                    nc.scalar.dma_start(bqt[:], bq_d[br])
                    nc.scalar.dma_start(wqt[:], Wq_d[br].rearrange("k p a -> p k a"))
                # per-k W2/W3 tiles: exact DMA deps, split across two queues
                w2ts = []
                for k in range(K2):
                    w2k = wp2.tile([128, D2], bf16, tag="w2", name=f"w2_{br}_{k}")
                    (nc.sync, nc.gpsimd)[k % 2].dma_start(w2k[:], W2_d[br, k])
                    w2ts.append(w2k)
                w3ts = []
                for k in range(K3):
                    w3k = wp3.tile([128, D3], bf16, tag="w3", name=f"w3_{br}_{k}")
                    (nc.gpsimd, nc.sync)[k % 2].dma_start(w3k[:], W3_d[br, k])
                    w3ts.append(w3k)
                loaded[br] = (w1t, w2ts, w3ts, wqt, btile, bqt)

            def load_px(idx):
                # px duplicated into partitions 64.. for L1 row packing
                br, bt = iters[idx]
                eng = nc.sync if idx == 0 else nc.scalar
                bsl = slice(bt * BT, (bt + 1) * BT)
                px = pxp.tile([128, BT], bf16, tag="px", name=f"px_{idx}")
                g0 = NODE + GRP * br
                eng.dma_start(px[0:NODE, :], xT_d[0:NODE, bsl])
                eng.dma_start(px[NODE:D0, :], xT_d[g0:g0 + GRP, bsl])
                eng.dma_start(px[64:64 + NODE, :], xT_d[0:NODE, bsl])
                eng.dma_start(px[64 + NODE:64 + D0, :], xT_d[g0:g0 + GRP, bsl])
                pxs[idx] = px

            def drain(dst, ps, bias, j):
                if j % 2 == 0:
                    nc.scalar.activation(dst, ps, Relu, bias=bias, scale=1.0)
                else:
                    nc.vector.tensor_scalar(dst, ps, bias, 0.0, ADD, MAX)

            def emit_L1_pair(idx, mp):
                br, _ = iters[idx]
                w1t = loaded[br][0]
                btile = loaded[br][4]
                px = pxs[idx]
                m0, m1 = 2 * mp, 2 * mp + 1
                psA = psp.tile([128, BT], f32, tag="ps", name=f"l1A_{idx}_{mp}")
                psB = psp.tile([128, BT], f32, tag="ps", name=f"l1B_{idx}_{mp}")
                nc.tensor.matmul(
                    psA[:], w1t[0:D0, m0 * 128:(m0 + 1) * 128], px[0:D0, :],
                    start=True, stop=True, tile_position=(0, 0),
                )
                nc.tensor.matmul(
                    psB[:], w1t[64:64 + D0, m1 * 128:(m1 + 1) * 128],
                    px[64:64 + D0, :],
                    start=True, stop=True, tile_position=(64, 0),
                )
                drain(h1[:, m0, :], psA[:], btile[:, m0:m0 + 1], m0)
                drain(h1[:, m1, :], psB[:], btile[:, m1:m1 + 1], m1)

            # prologue: px + small branch-0 tensors land first, ahead of the
            # W2/W3 bulk, so the first L1 matmul starts ~4us in
            load_px(0)
            load_branch(0)
            for mp in range(MP1):
                emit_L1_pair(0, mp)

            for idx, (br, bt) in enumerate(iters):
                w1t, w2ts, w3ts, wqt, btile, bqt = loaded[br]
                nxt = idx + 1
                if nxt < len(iters):
                    nbr = iters[nxt][0]
                    if nbr not in loaded:
                        load_branch(nbr)
                    load_px(nxt)

                # ---- L2: [2048 -> 1024], m-pair outer so each 2-bank group
                # finishes and drains early (spreads drain-engine load) ----
                for mp in range(M2 // 2):
                    ps2 = [psp.tile([128, BT], f32, tag="ps",
                                    name=f"ps2_{idx}_{mp}_{_i}")
                           for _i in range(2)]
                    for k in range(K2):
                        for mi in range(2):
                            m = 2 * mp + mi
                            nc.tensor.matmul(
                                ps2[mi][:], w2ts[k][:, m * 128:(m + 1) * 128],
                                h1[:, k, :],
                                start=(k == 0), stop=(k == K2 - 1),
                            )
                    for mi in range(2):
                        m = 2 * mp + mi
                        drain(h2[:, m, :], ps2[mi][:],
                              btile[:, M1 + m:M1 + m + 1], m)

                # ---- L3 [1024 -> 512] interleaved with next iteration's L1
                # (one row-packed pair per k step) ----
                ps3 = [psp.tile([128, BT], f32, tag="ps", name=f"ps3_{idx}_{_m}")
                       for _m in range(M3)]
                for k in range(K3):
                    for m in range(M3):
                        nc.tensor.matmul(
                            ps3[m][:], w3ts[k][:, m * 128:(m + 1) * 128],
                            h2[:, k, :],
                            start=(k == 0), stop=(k == K3 - 1),
                        )
                    if nxt < len(iters):
                        emit_L1_pair(nxt, k)
                for m in range(M3):
                    drain(h3[:, m, :], ps3[m][:],
                          btile[:, M1 + M2 + m:M1 + M2 + m + 1], m)

                # ---- head: q^T = Wq^T h3 + bq, action-major [12, BT] ----
                bsl = slice(bt * BT, (bt + 1) * BT)
                psh = psp.tile([NAP, BT], f32, tag="ps", name=f"psh_{idx}")
                for k in range(KH):
                    nc.tensor.matmul(
                        psh[:], wqt[:, k, :], h3[:, k, :],
                        start=(k == 0), stop=(k == KH - 1),
                    )
                ost = osp.tile([NAP, BT], f32, tag="os", name=f"ost_{idx}")
                nc.vector.tensor_scalar(ost[:], psh[:], bqt[:], 0.0, ADD)
                nc.sync.dma_start(out_d[br, :, bsl], ost[0:NA, :])

    nc.compile()
    _NC_CACHE["nc"] = nc
    return nc


def _pack_weights(W1, b1, W2, b2, W3, b3, Wv, bv, Wa, ba):
    f = np.float32
    # W1 duplicated into rows 64.. for tile_position row packing
    W1p = np.zeros((NB, 128, D1), BF16)
    W1p[:, 0:D0] = W1.astype(BF16)
    W1p[:, 64:64 + D0] = W1p[:, 0:D0]
    W2p = np.ascontiguousarray(W2.reshape(NB, K2, 128, D2).astype(BF16))
    W3p = np.ascontiguousarray(W3.reshape(NB, K3, 128, D3).astype(BF16))
    # fold dueling head: q = h @ (Wv + Wa - mean(Wa)) + (bv + ba - mean(ba))
    Wq = Wv + Wa - Wa.mean(axis=2, keepdims=True)                # [12, 512, 11]
    bq = bv + ba - ba.mean(axis=1, keepdims=True)                # [12, 11]
    Wq = np.concatenate([Wq, np.zeros((NB, D3, NAP - NA), Wq.dtype)], axis=2)
    bq = np.concatenate([bq, np.zeros((NB, NAP - NA), bq.dtype)], axis=1)
    Wqp = np.ascontiguousarray(Wq.reshape(NB, KH, 128, NAP).astype(BF16))
    bp = np.concatenate(
        [
            b1.reshape(NB, M1, 128).transpose(0, 2, 1),
            b2.reshape(NB, M2, 128).transpose(0, 2, 1),
            b3.reshape(NB, M3, 128).transpose(0, 2, 1),
        ],
        axis=2,
    ).astype(f)                                                  # [12, 128, 28]
    bqp = np.ascontiguousarray(bq[:, :, None], f)                # [12, 12, 1]
    return W1p, W2p, W3p, Wqp, bp, bqp


def kernel(x, W1, b1, W2, b2, W3, b3, Wv, bv, Wa, ba):
    global LAST_RESULT
    from concourse.bass_utils import run_bass_kernel_spmd

    x = np.asarray(x, np.float32)
    args = [np.asarray(a, np.float32) for a in (W1, b1, W2, b2, W3, b3, Wv, bv, Wa, ba)]
    W1p, W2p, W3p, Wqp, bp, bqp = _pack_weights(*args)

    nc = _build_nc()
    in_maps = []
    for c in range(NCORES):
        xT = np.ascontiguousarray(x[c * LB:(c + 1) * LB].T.astype(BF16))
        in_maps.append({
            "xT": xT,
            "W1p": W1p, "W2p": W2p, "W3p": W3p, "Wqp": Wqp,
            "bp": bp, "bqp": bqp,
        })

    res = run_bass_kernel_spmd(nc, in_maps, list(range(NCORES)))
    LAST_RESULT = res

    out = np.empty((NB, B, NA), np.float32)
    for c in range(NCORES):
        out[:, c * LB:(c + 1) * LB, :] = res.results[c]["out"].transpose(0, 2, 1)
    return out


# revision 13
# speedup vs baseline: 1.0143x; 1.0143x over previous
"""Trainium2 Bass kernel for nn_BranchingQNetwork (12-branch dueling Q-MLP).

Strategy: data-parallel over batch (8 cores x 1024 rows). Per core, all 12
branch MLPs run as feature-major GEMM chains in bf16 (weights stationary,
activations streaming, fp32 PSUM accumulate). bf16 weight tiles get the
fast-weight-load path (FWL), so each 128x128 LDWEIGHTS (~50-100ns) hides
behind its 512-column MATMUL (~216ns) — the fp32r baseline was
LDWEIGHTS-limited. Per-branch weights are SBUF-resident as per-k tiles
(prefetched one branch ahead, exact DMA deps), so W2 is read from HBM once
per branch instead of once per batch tile. Layer 1 has K=62 < 128, so its
m-tile pairs run concurrently in disjoint 64-row PE array groups
(tile_position row packing with px duplicated into partitions 64..125),
halving L1 column time. The dueling head (v + a - mean(a)) is linear and
folded into a single [512, 11] matrix on the host; it runs with Wq
stationary producing [11, batch]-major output, transposed on the host.
"""
import sys

sys.path.insert(0, "/opt/trn_rl_repo")

import numpy as np
import ml_dtypes

# problem dims (hardcoded per harness contract)
B = 8192
OBS = 249
NB = 12
NA = 11
NODE = 45
GRP = 17
D0 = 62
D1 = 2048
D2 = 1024
D3 = 512

NCORES = 8
LB = B // NCORES     # local batch per core
BT = 512             # batch tile
NBT = LB // BT
M1 = D1 // 128       # 16 output tiles of layer 1
MP1 = M1 // 2        # 8 row-packed L1 pairs
K2 = D1 // 128       # 16 contraction tiles of layer 2
M2 = D2 // 128       # 8
K3 = D2 // 128       # 8
M3 = D3 // 128       # 4
KH = D3 // 128       # 4
NAP = 12             # head width padded even

BF16 = ml_dtypes.bfloat16

_NC_CACHE = {}
LAST_RESULT = None


def _build_nc():
    if "nc" in _NC_CACHE:
        return _NC_CACHE["nc"]
    from concourse import bacc
    import concourse.mybir as mybir
    import concourse.tile as tile

    f32 = mybir.dt.float32
    bf16 = mybir.dt.bfloat16
    Relu = mybir.ActivationFunctionType.Relu
    ADD = mybir.AluOpType.add
    MAX = mybir.AluOpType.max

    nc = bacc.Bacc("TRN2")

    xT_d = nc.declare_dram_parameter("xT", [OBS, LB], bf16, isOutput=False)
    W1_d = nc.declare_dram_parameter("W1p", [NB, 128, D1], bf16, isOutput=False)
    W2_d = nc.declare_dram_parameter("W2p", [NB, K2, 128, D2], bf16, isOutput=False)
    W3_d = nc.declare_dram_parameter("W3p", [NB, K3, 128, D3], bf16, isOutput=False)
    Wq_d = nc.declare_dram_parameter("Wqp", [NB, KH, 128, NAP], bf16, isOutput=False)
    b_d = nc.declare_dram_parameter("bp", [NB, 128, M1 + M2 + M3], f32, isOutput=False)
    bq_d = nc.declare_dram_parameter("bqp", [NB, NAP, 1], f32, isOutput=False)
    out_d = nc.declare_dram_parameter("out", [NB, NA, LB], f32, isOutput=True)

    with tile.TileContext(nc) as tc:
        with (
            tc.tile_pool(name="wp1", bufs=2) as wp1,
            tc.tile_pool(name="wp2", bufs=2 * K2) as wp2,
            tc.tile_pool(name="wp3", bufs=2 * K3) as wp3,
            tc.tile_pool(name="wpq", bufs=2) as wpq,
            tc.tile_pool(name="bbp", bufs=2) as bbp,
            tc.tile_pool(name="pxp", bufs=3) as pxp,
            tc.tile_pool(name="actp", bufs=1) as actp,
            tc.tile_pool(name="osp", bufs=3) as osp,
            tc.tile_pool(name="psp", bufs=8, space="PSUM") as psp,
        ):
            h1 = actp.tile([128, K2, BT], bf16, tag="h1")
            h2 = actp.tile([128, K3, BT], bf16, tag="h2")
            h3 = actp.tile([128, KH, BT], bf16, tag="h3")

            iters = [(br, bt) for br in range(NB) for bt in range(NBT)]
            loaded = {}
            pxs = {}

            def load_branch(br):
                w1t = wp1.tile([128, D1], bf16, tag="w1", name=f"w1_{br}")
                btile = bbp.tile([128, M1 + M2 + M3], f32, tag="b", name=f"b_{br}")
                bqt = bbp.tile([NAP, 1], f32, tag="bq", name=f"bq_{br}")
                wqt = wpq.tile([128, KH, NAP], bf16, tag="wq", name=f"wq_{br}")
                if br == 0:
                    # chunk W1[0] across both queues so the first L1 pair can
                    # start as soon as px + the first 512 columns have landed
                    nc.scalar.dma_start(btile[:], b_d[br])
                    nc.scalar.dma_start(bqt[:], bq_d[br])
                    nc.scalar.dma_start(wqt[:], Wq_d[br].rearrange("k p a -> p k a"))
                    for c in range(4):
                        eng = (nc.sync, nc.scalar)[c % 2]
                        csl = slice(c * 512, (c + 1) * 512)
                        eng.dma_start(w1t[:, csl], W1_d[br, :, csl])
                else:
                    nc.scalar.dma_start(w1t[:], W1_d[br])
                    nc.scalar.dma_start(btile[:], b_d[br])
                    nc.scalar.dma_start(bqt[:], bq_d[br])
                    nc.scalar.dma_start(wqt[:], Wq_d[br].rearrange("k p a -> p k a"))
                # per-k W2/W3 tiles: exact DMA deps, single ring (two rings
                # bursting together contend with PE operand reads in SBUF)
                w2ts = []
                for k in range(K2):
                    w2k = wp2.tile([128, D2], bf16, tag="w2", name=f"w2_{br}_{k}")
                    nc.sync.dma_start(w2k[:], W2_d[br, k])
                    w2ts.append(w2k)
                w3ts = []
                for k in range(K3):
                    w3k = wp3.tile([128, D3], bf16, tag="w3", name=f"w3_{br}_{k}")
                    nc.sync.dma_start(w3k[:], W3_d[br, k])
                    w3ts.append(w3k)
                loaded[br] = (w1t, w2ts, w3ts, wqt, btile, bqt)

            def load_px(idx):
                # px duplicated into partitions 64.. for L1 row packing
                br, bt = iters[idx]
                eng = nc.sync if idx == 0 else nc.scalar
                bsl = slice(bt * BT, (bt + 1) * BT)
                px = pxp.tile([128, BT], bf16, tag="px", name=f"px_{idx}")
                g0 = NODE + GRP * br
                eng.dma_start(px[0:NODE, :], xT_d[0:NODE, bsl])
                eng.dma_start(px[NODE:D0, :], xT_d[g0:g0 + GRP, bsl])
                eng.dma_start(px[64:64 + NODE, :], xT_d[0:NODE, bsl])
                eng.dma_start(px[64 + NODE:64 + D0, :], xT_d[g0:g0 + GRP, bsl])
                pxs[idx] = px

            def drain(dst, ps, bias, j):
                if j % 2 == 0:
                    nc.scalar.activation(dst, ps, Relu, bias=bias, scale=1.0)
                else:
                    nc.vector.tensor_scalar(dst, ps, bias, 0.0, ADD, MAX)

            def emit_L1_pair(idx, mp):
                br, _ = iters[idx]
                w1t = loaded[br][0]
                btile = loaded[br][4]
                px = pxs[idx]
                m0, m1 = 2 * mp, 2 * mp + 1
                psA = psp.tile([128, BT], f32, tag="ps", name=f"l1A_{idx}_{mp}")
                psB = psp.tile([128, BT], f32, tag="ps", name=f"l1B_{idx}_{mp}")
                nc.tensor.matmul(
                    psA[:], w1t[0:D0, m0 * 128:(m0 + 1) * 128], px[0:D0, :],
                    start=True, stop=True, tile_position=(0, 0),
                )
                nc.tensor.matmul(
                    psB[:], w1t[64:64 + D0, m1 * 128:(m1 + 1) * 128],
                    px[64:64 + D0, :],
                    start=True, stop=True, tile_position=(64, 0),
                )
                drain(h1[:, m0, :], psA[:], btile[:, m0:m0 + 1], m0)
                drain(h1[:, m1, :], psB[:], btile[:, m1:m1 + 1], m1)

            # prologue: px + small branch-0 tensors land first, ahead of the
            # W2/W3 bulk, so the first L1 matmul starts ~4us in
            load_px(0)
            load_branch(0)
            for mp in range(MP1):
                emit_L1_pair(0, mp)

            for idx, (br, bt) in enumerate(iters):
                w1t, w2ts, w3ts, wqt, btile, bqt = loaded[br]
                nxt = idx + 1
                if nxt < len(iters):
                    nbr = iters[nxt][0]
                    if nbr not in loaded:
                        load_branch(nbr)
                    load_px(nxt)

                # ---- L2: [2048 -> 1024], k-outer, 8 psum banks ----
                ps2 = [psp.tile([128, BT], f32, tag="ps", name=f"ps2_{idx}_{_m}")
                       for _m in range(M2)]
                for k in range(K2):
                    for m in range(M2):
                        nc.tensor.matmul(
                            ps2[m][:], w2ts[k][:, m * 128:(m + 1) * 128],
                            h1[:, k, :],
                            start=(k == 0), stop=(k == K2 - 1),
                        )
                for m in range(M2):
                    drain(h2[:, m, :], ps2[m][:], btile[:, M1 + m:M1 + m + 1], m)

                # ---- L3 [1024 -> 512] interleaved with next iteration's L1
                # (one row-packed pair per k step) ----
                ps3 = [psp.tile([128, BT], f32, tag="ps", name=f"ps3_{idx}_{_m}")
                       for _m in range(M3)]
                for k in range(K3):
                    for m in range(M3):
                        nc.tensor.matmul(
                            ps3[m][:], w3ts[k][:, m * 128:(m + 1) * 128],
                            h2[:, k, :],
                            start=(k == 0), stop=(k == K3 - 1),
                        )
                    if nxt < len(iters):
                        emit_L1_pair(nxt, k)
                for m in range(M3):
                    drain(h3[:, m, :], ps3[m][:],
                          btile[:, M1 + M2 + m:M1 + M2 + m + 1], m)

                # ---- head: q^T = Wq^T h3 + bq, action-major [12, BT] ----
                bsl = slice(bt * BT, (bt + 1) * BT)
                psh = psp.tile([NAP, BT], f32, tag="ps", name=f"psh_{idx}")
                for k in range(KH):
                    nc.tensor.matmul(
                        psh[:], wqt[:, k, :], h3[:, k, :],
                        start=(k == 0), stop=(k == KH - 1),
                    )
                ost = osp.tile([NAP, BT], f32, tag="os", name=f"ost_{idx}")
                nc.vector.tensor_scalar(ost[:], psh[:], bqt[:], 0.0, ADD)
                nc.sync.dma_start(out_d[br, :, bsl], ost[0:NA, :])

    nc.compile()
    _NC_CACHE["nc"] = nc
    return nc


def _pack_weights(W1, b1, W2, b2, W3, b3, Wv, bv, Wa, ba):
    f = np.float32
    # W1 duplicated into rows 64.. for tile_position row packing
    W1p = np.zeros((NB, 128, D1), BF16)
    W1p[:, 0:D0] = W1.astype(BF16)
    W1p[:, 64:64 + D0] = W1p[:, 0:D0]
    W2p = np.ascontiguousarray(W2.reshape(NB, K2, 128, D2).astype(BF16))
    W3p = np.ascontiguousarray(W3.reshape(NB, K3, 128, D3).astype(BF16))
    # fold dueling head: q = h @ (Wv + Wa - mean(Wa)) + (bv + ba - mean(ba))
    Wq = Wv + Wa - Wa.mean(axis=2, keepdims=True)                # [12, 512, 11]
    bq = bv + ba - ba.mean(axis=1, keepdims=True)                # [12, 11]
    Wq = np.concatenate([Wq, np.zeros((NB, D3, NAP - NA), Wq.dtype)], axis=2)
    bq = np.concatenate([bq, np.zeros((NB, NAP - NA), bq.dtype)], axis=1)
    Wqp = np.ascontiguousarray(Wq.reshape(NB, KH, 128, NAP).astype(BF16))
    bp = np.concatenate(
        [
            b1.reshape(NB, M1, 128).transpose(0, 2, 1),
            b2.reshape(NB, M2, 128).transpose(0, 2, 1),
            b3.reshape(NB, M3, 128).transpose(0, 2, 1),
        ],
        axis=2,
    ).astype(f)                                                  # [12, 128, 28]
    bqp = np.ascontiguousarray(bq[:, :, None], f)                # [12, 12, 1]
    return W1p, W2p, W3p, Wqp, bp, bqp


def kernel(x, W1, b1, W2, b2, W3, b3, Wv, bv, Wa, ba):
    global LAST_RESULT
    from concourse.bass_utils import run_bass_kernel_spmd

    x = np.asarray(x, np.float32)
    args = [np.asarray(a, np.float32) for a in (W1, b1, W2, b2, W3, b3, Wv, bv, Wa, ba)]
    W1p, W2p, W3p, Wqp, bp, bqp = _pack_weights(*args)

    nc = _build_nc()
    in_maps = []
    for c in range(NCORES):
        xT = np.ascontiguousarray(x[c * LB:(c + 1) * LB].T.astype(BF16))
        in_maps.append({
            "xT": xT,
            "W1p": W1p, "W2p": W2p, "W3p": W3p, "Wqp": Wqp,
            "bp": bp, "bqp": bqp,
        })

    res = run_bass_kernel_spmd(nc, in_maps, list(range(NCORES)))
    LAST_RESULT = res

    out = np.empty((NB, B, NA), np.float32)
    for c in range(NCORES):
        out[:, c * LB:(c + 1) * LB, :] = res.results[c]["out"].transpose(0, 2, 1)
    return out


# revision 15
# speedup vs baseline: 1.0157x; 1.0013x over previous
"""Trainium2 Bass kernel for nn_BranchingQNetwork (12-branch dueling Q-MLP).

Strategy: data-parallel over batch (8 cores x 1024 rows). Per core, all 12
branch MLPs run as feature-major GEMM chains in bf16 (weights stationary,
activations streaming, fp32 PSUM accumulate). bf16 weight tiles get the
fast-weight-load path (FWL), so each 128x128 LDWEIGHTS (~50-100ns) hides
behind its 512-column MATMUL (~216ns) — the fp32r baseline was
LDWEIGHTS-limited. Per-branch weights are SBUF-resident as per-k tiles
(prefetched one branch ahead, exact DMA deps), so W2 is read from HBM once
per branch instead of once per batch tile. Layer 1 has K=62 < 128, so its
m-tile pairs run concurrently in disjoint 64-row PE array groups
(tile_position row packing with px duplicated into partitions 64..125),
halving L1 column time. The dueling head (v + a - mean(a)) is linear and
folded into a single [512, 11] matrix on the host; it runs with Wq
stationary producing [11, batch]-major output, transposed on the host.
"""
import sys

sys.path.insert(0, "/opt/trn_rl_repo")

import numpy as np
import ml_dtypes

# problem dims (hardcoded per harness contract)
B = 8192
OBS = 249
NB = 12
NA = 11
NODE = 45
GRP = 17
D0 = 62
D1 = 2048
D2 = 1024
D3 = 512

NCORES = 8
LB = B // NCORES     # local batch per core
BT = 512             # batch tile
NBT = LB // BT
M1 = D1 // 128       # 16 output tiles of layer 1
MP1 = M1 // 2        # 8 row-packed L1 pairs
K2 = D1 // 128       # 16 contraction tiles of layer 2
M2 = D2 // 128       # 8
K3 = D2 // 128       # 8
M3 = D3 // 128       # 4
KH = D3 // 128       # 4
NAP = 12             # head width padded even

BF16 = ml_dtypes.bfloat16

_NC_CACHE = {}
LAST_RESULT = None


def _build_nc():
    if "nc" in _NC_CACHE:
        return _NC_CACHE["nc"]
    from concourse import bacc
    import concourse.mybir as mybir
    import concourse.tile as tile

    f32 = mybir.dt.float32
    bf16 = mybir.dt.bfloat16
    Relu = mybir.ActivationFunctionType.Relu
    ADD = mybir.AluOpType.add
    MAX = mybir.AluOpType.max

    nc = bacc.Bacc("TRN2")

    xT_d = nc.declare_dram_parameter("xT", [OBS, LB], bf16, isOutput=False)
    W1_d = nc.declare_dram_parameter("W1p", [NB, 128, D1], bf16, isOutput=False)
    W2_d = nc.declare_dram_parameter("W2p", [NB, K2, 128, D2], bf16, isOutput=False)
    W3_d = nc.declare_dram_parameter("W3p", [NB, K3, 128, D3], bf16, isOutput=False)
    Wq_d = nc.declare_dram_parameter("Wqp", [NB, KH, 128, NAP], bf16, isOutput=False)
    b_d = nc.declare_dram_parameter("bp", [NB, 128, M1 + M2 + M3], f32, isOutput=False)
    bq_d = nc.declare_dram_parameter("bqp", [NB, NAP, 1], f32, isOutput=False)
    out_d = nc.declare_dram_parameter("out", [NB, NA, LB], f32, isOutput=True)

    with tile.TileContext(nc) as tc:
        with (
            tc.tile_pool(name="wp1", bufs=2) as wp1,
            tc.tile_pool(name="wp2", bufs=K2) as wp2,
            tc.tile_pool(name="wp3", bufs=2 * K3) as wp3,
            tc.tile_pool(name="wpq", bufs=2) as wpq,
            tc.tile_pool(name="bbp", bufs=2) as bbp,
            tc.tile_pool(name="pxp", bufs=3) as pxp,
            tc.tile_pool(name="actp", bufs=1) as actp,
            tc.tile_pool(name="osp", bufs=3) as osp,
            tc.tile_pool(name="psp", bufs=8, space="PSUM") as psp,
        ):
            h1 = actp.tile([128, K2, BT], bf16, tag="h1")
            h2 = actp.tile([128, K3, BT], bf16, tag="h2")
            h3 = actp.tile([128, KH, BT], bf16, tag="h3")

            iters = [(br, bt) for br in range(NB) for bt in range(NBT)]
            loaded = {}
            pxs = {}

            def load_branch(br):
                w1t = wp1.tile([128, D1], bf16, tag="w1", name=f"w1_{br}")
                btile = bbp.tile([128, M1 + M2 + M3], f32, tag="b", name=f"b_{br}")
                bqt = bbp.tile([NAP, 1], f32, tag="bq", name=f"bq_{br}")
                wqt = wpq.tile([128, KH, NAP], bf16, tag="wq", name=f"wq_{br}")
                if br == 0:
                    # chunk W1[0] across scalar+gpsimd (px has sync) so the
                    # first L1 pair starts as soon as px + chunk0 land
                    for c in range(4):
                        eng = (nc.scalar, nc.gpsimd)[c // 2]
                        csl = slice(c * 512, (c + 1) * 512)
                        eng.dma_start(w1t[:, csl], W1_d[br, :, csl])
                    nc.scalar.dma_start(btile[:], b_d[br])
                    nc.scalar.dma_start(bqt[:], bq_d[br])
                    nc.scalar.dma_start(wqt[:], Wq_d[br].rearrange("k p a -> p k a"))
                else:
                    nc.scalar.dma_start(w1t[:], W1_d[br])
                    nc.scalar.dma_start(btile[:], b_d[br])
                    nc.scalar.dma_start(bqt[:], bq_d[br])
                    nc.scalar.dma_start(wqt[:], Wq_d[br].rearrange("k p a -> p k a"))
                # per-k W2/W3 tiles: exact DMA deps, single ring (two rings
                # bursting together contend with PE operand reads in SBUF)
                w2ts = []
                for k in range(K2):
                    w2k = wp2.tile([128, D2], bf16, tag="w2", name=f"w2_{br}_{k}")
                    nc.sync.dma_start(w2k[:], W2_d[br, k])
                    w2ts.append(w2k)
                w3ts = []
                for k in range(K3):
                    w3k = wp3.tile([128, D3], bf16, tag="w3", name=f"w3_{br}_{k}")
                    nc.sync.dma_start(w3k[:], W3_d[br, k])
                    w3ts.append(w3k)
                loaded[br] = (w1t, w2ts, w3ts, wqt, btile, bqt)

            def load_px(idx):
                # px duplicated into partitions 64.. for L1 row packing
                br, bt = iters[idx]
                eng = nc.sync if idx == 0 else nc.scalar
                bsl = slice(bt * BT, (bt + 1) * BT)
                px = pxp.tile([128, BT], bf16, tag="px", name=f"px_{idx}")
                g0 = NODE + GRP * br
                eng.dma_start(px[0:NODE, :], xT_d[0:NODE, bsl])
                eng.dma_start(px[NODE:D0, :], xT_d[g0:g0 + GRP, bsl])
                eng.dma_start(px[64:64 + NODE, :], xT_d[0:NODE, bsl])
                eng.dma_start(px[64 + NODE:64 + D0, :], xT_d[g0:g0 + GRP, bsl])
                pxs[idx] = px

            def drain(dst, ps, bias, j):
                if j % 2 == 0:
                    nc.scalar.activation(dst, ps, Relu, bias=bias, scale=1.0)
                else:
                    nc.vector.tensor_scalar(dst, ps, bias, 0.0, ADD, MAX)

            def emit_L1_pair(idx, mp):
                br, _ = iters[idx]
                w1t = loaded[br][0]
                btile = loaded[br][4]
                px = pxs[idx]
                m0, m1 = 2 * mp, 2 * mp + 1
                psA = psp.tile([128, BT], f32, tag="ps", name=f"l1A_{idx}_{mp}")
                psB = psp.tile([128, BT], f32, tag="ps", name=f"l1B_{idx}_{mp}")
                nc.tensor.matmul(
                    psA[:], w1t[0:D0, m0 * 128:(m0 + 1) * 128], px[0:D0, :],
                    start=True, stop=True, tile_position=(0, 0),
                )
                nc.tensor.matmul(
                    psB[:], w1t[64:64 + D0, m1 * 128:(m1 + 1) * 128],
                    px[64:64 + D0, :],
                    start=True, stop=True, tile_position=(64, 0),
                )
                drain(h1[:, m0, :], psA[:], btile[:, m0:m0 + 1], m0)
                drain(h1[:, m1, :], psB[:], btile[:, m1:m1 + 1], m1)

            # prologue: px + small branch-0 tensors land first, ahead of the
            # W2/W3 bulk, so the first L1 matmul starts ~4us in
            load_px(0)
            load_branch(0)
            for mp in range(MP1):
                emit_L1_pair(0, mp)

            for idx, (br, bt) in enumerate(iters):
                w1t, w2ts, w3ts, wqt, btile, bqt = loaded[br]
                nxt = idx + 1
                if nxt < len(iters):
                    nbr = iters[nxt][0]
                    if nbr not in loaded:
                        load_branch(nbr)
                    load_px(nxt)

                # ---- L2: [2048 -> 1024], k-outer, 8 psum banks ----
                ps2 = [psp.tile([128, BT], f32, tag="ps", name=f"ps2_{idx}_{_m}")
                       for _m in range(M2)]
                for k in range(K2):
                    for m in range(M2):
                        nc.tensor.matmul(
                            ps2[m][:], w2ts[k][:, m * 128:(m + 1) * 128],
                            h1[:, k, :],
                            start=(k == 0), stop=(k == K2 - 1),
                        )
                for m in range(M2):
                    drain(h2[:, m, :], ps2[m][:], btile[:, M1 + m:M1 + m + 1], m)

                # ---- L3 [1024 -> 512] interleaved with next iteration's L1
                # (one row-packed pair per k step) ----
                ps3 = [psp.tile([128, BT], f32, tag="ps", name=f"ps3_{idx}_{_m}")
                       for _m in range(M3)]
                for k in range(K3):
                    for m in range(M3):
                        nc.tensor.matmul(
                            ps3[m][:], w3ts[k][:, m * 128:(m + 1) * 128],
                            h2[:, k, :],
                            start=(k == 0), stop=(k == K3 - 1),
                        )
                    if nxt < len(iters):
                        emit_L1_pair(nxt, k)
                for m in range(M3):
                    drain(h3[:, m, :], ps3[m][:],
                          btile[:, M1 + M2 + m:M1 + M2 + m + 1], m)

                # ---- head: q^T = Wq^T h3 + bq, action-major [12, BT] ----
                bsl = slice(bt * BT, (bt + 1) * BT)
                psh = psp.tile([NAP, BT], f32, tag="ps", name=f"psh_{idx}")
                for k in range(KH):
                    nc.tensor.matmul(
                        psh[:], wqt[:, k, :], h3[:, k, :],
                        start=(k == 0), stop=(k == KH - 1),
                    )
                ost = osp.tile([NAP, BT], f32, tag="os", name=f"ost_{idx}")
                nc.vector.tensor_scalar(ost[:], psh[:], bqt[:], 0.0, ADD)
                nc.sync.dma_start(out_d[br, :, bsl], ost[0:NA, :])

    nc.compile()
    _NC_CACHE["nc"] = nc
    return nc


def _pack_weights(W1, b1, W2, b2, W3, b3, Wv, bv, Wa, ba):
    f = np.float32
    # W1 duplicated into rows 64.. for tile_position row packing
    W1p = np.zeros((NB, 128, D1), BF16)
    W1p[:, 0:D0] = W1.astype(BF16)
    W1p[:, 64:64 + D0] = W1p[:, 0:D0]
    W2p = np.ascontiguousarray(W2.reshape(NB, K2, 128, D2).astype(BF16))
    W3p = np.ascontiguousarray(W3.reshape(NB, K3, 128, D3).astype(BF16))
    # fold dueling head: q = h @ (Wv + Wa - mean(Wa)) + (bv + ba - mean(ba))
    Wq = Wv + Wa - Wa.mean(axis=2, keepdims=True)                # [12, 512, 11]
    bq = bv + ba - ba.mean(axis=1, keepdims=True)                # [12, 11]
    Wq = np.concatenate([Wq, np.zeros((NB, D3, NAP - NA), Wq.dtype)], axis=2)
    bq = np.concatenate([bq, np.zeros((NB, NAP - NA), bq.dtype)], axis=1)
    Wqp = np.ascontiguousarray(Wq.reshape(NB, KH, 128, NAP).astype(BF16))
    bp = np.concatenate(
        [
            b1.reshape(NB, M1, 128).transpose(0, 2, 1),
            b2.reshape(NB, M2, 128).transpose(0, 2, 1),
            b3.reshape(NB, M3, 128).transpose(0, 2, 1),
        ],
        axis=2,
    ).astype(f)                                                  # [12, 128, 28]
    bqp = np.ascontiguousarray(bq[:, :, None], f)                # [12, 12, 1]
    return W1p, W2p, W3p, Wqp, bp, bqp


def kernel(x, W1, b1, W2, b2, W3, b3, Wv, bv, Wa, ba):
    global LAST_RESULT
    from concourse.bass_utils import run_bass_kernel_spmd

    x = np.asarray(x, np.float32)
    args = [np.asarray(a, np.float32) for a in (W1, b1, W2, b2, W3, b3, Wv, bv, Wa, ba)]
    W1p, W2p, W3p, Wqp, bp, bqp = _pack_weights(*args)

    nc = _build_nc()
    in_maps = []
    for c in range(NCORES):
        xT = np.ascontiguousarray(x[c * LB:(c + 1) * LB].T.astype(BF16))
        in_maps.append({
            "xT": xT,
            "W1p": W1p, "W2p": W2p, "W3p": W3p, "Wqp": Wqp,
            "bp": bp, "bqp": bqp,
        })

    res = run_bass_kernel_spmd(nc, in_maps, list(range(NCORES)))
    LAST_RESULT = res

    out = np.empty((NB, B, NA), np.float32)
    for c in range(NCORES):
        out[:, c * LB:(c + 1) * LB, :] = res.results[c]["out"].transpose(0, 2, 1)
    return out
